# revision 29
# baseline (speedup 1.0000x reference)
"""Trainium2 Bass kernel for nn_Loss_2482491097912 (SimCLR-style semi-supervised loss).

Strategy (8 NeuronCores, data-parallel over anchor rows):
  - Each core receives the FULL z1/z2 (pre-transposed + bf16 on host) and its
    1024-row slice of the masks, with h-columns ROLLED so that every core's
    local rows sit at columns [0:1024] -> one shared SPMD program.
  - On-core: projection MLP in transposed layout (HT = [feat, rows]),
    column-normalize (1/||h|| via Ln/Exp), then sim matmuls + exp with fused
    row-sum accumulation, masked sums via DVE tensor ops.
  - v2 improvements over the first working version:
      * S21 is never computed: its row sums are column sums of E12 (PSUM-
        accumulated ones-matmuls) and its masked sums use host-transposed
        masks applied to E12.
      * S11/S22 exploit symmetry: only 5 of 8 rolled column blocks are
        computed per core (cyclic triangle cover); the missing row-sum
        pieces come from column sums of blocks 1..3.
      * exp output and masked-sum operands are bf16 (2x DVE throughput);
        masks are shipped as bf16 from host.
      * activation-table steering: all scalar-engine functions resolve to
        the one table set that holds Exp+Ln+Square, removing ~65 table
        loads (~2.7us each) from the critical path.
  - Host combines per-core row sums / col sums / masked sums / diag dots
    into the two scalar losses (cheap numpy tail math).
"""

import numpy as np
import ml_dtypes

import concourse.bass as bass
import concourse.bacc as bacc
import concourse.tile as tile
import concourse.mybir as mybir
from concourse.bass_utils import run_bass_kernel_spmd

F32 = mybir.dt.float32
BF16 = mybir.dt.bfloat16
FP8 = mybir.dt.float8e4
AF = mybir.ActivationFunctionType
OP = mybir.AluOpType
AX = mybir.AxisListType
DR = mybir.MatmulPerfMode.DoubleRow

# normalized H is stored as fp8e4m3 scaled by HSCALE (entries ~N(0, 0.7));
# sim matmuls then carry HSCALE^2, removed in the exp input scale.
HSCALE = 16.0
LN_HSCALE = float(np.log(HSCALE))
EXP_SCALE = 2.0 / (HSCALE * HSCALE)

N = 8192
D = 512
NCORES = 8
RPC = N // NCORES          # rows per core = 1024
NBLK = RPC // 128          # row blocks per core = 8
NH = 2 * N                 # 16384 stacked rows (h1 then h2)
PCH = 512                  # projection column chunk
NPCH = NH // PCH           # 32 projection chunks
CCH = 1024                 # phase-2 sim column chunk
NC12 = N // CCH            # 8 column chunks for S12
NCTRI = 5                  # 5 rolled column blocks for S11/S22 (triangle)
CSUM_TRI = (1, 2, 3)       # triangle chunks whose col sums we need

# acc_all rowsum/masked-sum column map (one f32 col per (sim, chunk, block)).
COL_S12 = 0                            # 8c x 8b = 64
COL_S11 = 64                           # 5c x 8b = 40
COL_S22 = 104                          # 40
COL_P12 = 144                          # 64  (E12 * pos)
COL_N12 = 208                          # 64  (E12 * neg)
COL_P21 = 272                          # 64  (E12 * pos^T)
COL_N21 = 336                          # 64  (E12 * neg^T)
ACC_COLS = 400

# csum output layout: [1, 14336] f32
CS_S12 = 0                 # 8192: colsums of E12 (rolled cols)
CS_S11 = N                 # 3072: colsums of E11 rolled chunks 1..3
CS_S22 = N + 3072          # 3072
CSUM_COLS = N + 6144


def _patch_act_tables():
    """Steer the act-table-load pass to the one set that holds Exp+Ln+Square
    (+Identity/Copy) so phase 1 never switches table sets."""
    import functools
    import concourse.hw_specs as hw_specs

    if getattr(bacc.get_activation_tables, "_steered", False):
        return
    orig = hw_specs.get_activation_tables
    keep_funcs = {AF.Exp, AF.Ln, AF.Square, AF.Identity, AF.Copy}

    @functools.cache
    def steered(arch):
        tabs = orig(arch)
        best = None
        for name, s in tabs.items():
            if keep_funcs <= s:
                best = name
                break
        if best is None:
            return tabs
        return {
            name: (s if name == best else (s - keep_funcs))
            for name, s in tabs.items()
        }

    steered._steered = True
    bacc.get_activation_tables = steered
    hw_specs.get_activation_tables = steered


def _emit(nc, tc, reps=1):
    # z / W1 / W2 in fp8, DoubleRow-interleaved [128, j, i, cols]
    # (row (2j+i)*128+p of the k-major matrix); W2 and b2 carry x16 which the
    # ln/exp normalization absorbs exactly.
    zp = nc.dram_tensor("zp", [128, 2, 2, NH], FP8, kind="ExternalInput").ap()
    w1p = nc.dram_tensor("w1p", [128, 2, 2, D], FP8, kind="ExternalInput").ap()
    b1r = nc.dram_tensor("b1r", [1, D], BF16, kind="ExternalInput").ap()
    w2p = nc.dram_tensor("w2p", [128, 2, 2, D], FP8, kind="ExternalInput").ap()
    b2r = nc.dram_tensor("b2r", [1, D], BF16, kind="ExternalInput").ap()
    # packed masks: value = pos + 2*neg in bf16 ({0,1,2,3}); pos recovered on
    # device via is_ge(1.5) with pk=2*pos+neg ordering -> see stt ops below.
    pk12 = nc.dram_tensor("pk12", [RPC, N], BF16, kind="ExternalInput").ap()
    pk21 = nc.dram_tensor("pk21", [RPC, N], BF16, kind="ExternalInput").ap()

    acc_out = nc.dram_tensor("acc", [128, ACC_COLS], F32, kind="ExternalOutput").ap()
    csum_out = nc.dram_tensor("csum", [1, CSUM_COLS], F32, kind="ExternalOutput").ap()
    d12_out = nc.dram_tensor("d12p", [1, RPC], F32, kind="ExternalOutput").ap()
    for _ in range(reps):
        _emit_body(nc, tc, zp, w1p, b1r, w2p, b2r, pk12, pk21,
                   acc_out, csum_out, d12_out)


def _emit_body(nc, tc, zp, w1p, b1r, w2p, b2r, pk12, pk21,
               acc_out, csum_out, d12_out):

    top = tc.alloc_tile_pool(name="top", bufs=1)
    # resident normalized H^T in fp8 (x HSCALE), DoubleRow-interleaved:
    # hp[j][:, i, col] holds k-tile (2j+i); [128, 2, 16384] = 32KB/part each
    hp = [top.tile([128, 2, NH], FP8, name=f"hp{j}", tag=f"hp{j}")
          for j in range(2)]
    acc_all = top.tile([128, ACC_COLS], F32, name="acc_all")
    w1_sb = top.tile([128, 2, 2, D], FP8, name="w1_sb")
    b1_sb = top.tile([1, D], BF16, name="b1_sb")
    w2_sb = top.tile([128, 2, 2, D], FP8, name="w2_sb")
    b2_sb = top.tile([1, D], BF16, name="b2_sb")
    ones_r = top.tile([1, D], BF16, name="ones_r")      # rhs for bias matmuls
    ones_cb = top.tile([128, 1], BF16, name="ones_cb")  # bf16 lhsT for col sums
    lnh_sb = top.tile([1, 1], F32, name="lnh_sb")       # ln(HSCALE) bias

    nc.sync.dma_start(w1_sb[:], w1p)
    nc.sync.dma_start(b1_sb[:], b1r)
    nc.sync.dma_start(w2_sb[:], w2p)
    nc.sync.dma_start(b2_sb[:], b2r)
    nc.vector.memset(ones_r[:], 1.0)
    nc.vector.memset(lnh_sb[:], LN_HSCALE)
    nc.vector.memset(ones_cb[:], 1.0)
    nc.vector.memset(acc_all[:], 0.0)

    # ---------------- Phase 1: projection (transposed layout) ----------------
    with (
        tc.tile_pool(name="pj_sb", bufs=2) as pj,
        tc.tile_pool(name="pp_l1", bufs=2, space="PSUM") as pp_l1,
        tc.tile_pool(name="pp_l2", bufs=1, space="PSUM") as pp_l2,
        tc.tile_pool(name="pp_n", bufs=2, space="PSUM") as pp_n,
    ):
        def norm_tail(cs, sqs, l2s):
            # deferred by one chunk: norms matmuls run behind the next
            # chunk's L1 so the PE never waits on the Square activations,
            # and the Ln/Exp/broadcast/normalize chain overlaps next L1/L2.
            def emit():
                norms_ps = pp_n.tile([1, PCH], F32, name="norms_ps", tag="n")
                for m in range(4):
                    nc.tensor.matmul(
                        norms_ps[:], ones_cb[:], sqs[m][:],
                        start=(m == 0), stop=(m == 3))
                # r = HSCALE/sqrt(n2) = exp(-0.5 * ln(n2) + ln HSCALE)
                lg = pj.tile([1, PCH], F32, name="lg", tag="lg")
                nc.scalar.activation(lg[:], norms_ps[:], AF.Ln)
                r1 = pj.tile([1, PCH], F32, name="r1", tag="r1")
                nc.scalar.activation(r1[:], lg[:], AF.Exp, scale=-0.5,
                                     bias=lnh_sb[:])
                rb = pj.tile([128, PCH], F32, name="rb", tag="rb")
                nc.gpsimd.partition_broadcast(rb[:], r1[:])
                for m in range(4):
                    # htn = (h + b2) * r -> fp8 into resident HP
                    nc.vector.tensor_mul(
                        hp[m // 2][:, m % 2, cs:cs + PCH], l2s[m][:], rb[:])
            return emit

        tail = None
        for c in range(NPCH):
            cs = c * PCH
            zt_t = pj.tile([128, 2, 2, PCH], FP8, name="zt_t", tag="zt")
            nc.sync.dma_start(zt_t[:], zp[:, :, :, cs:cs + PCH])

            gtp = [pj.tile([128, 2, PCH], FP8, name=f"gtp{j}", tag=f"gtp{j}")
                   for j in range(2)]
            for m in range(4):
                ms = m * 128
                l1_ps = pp_l1.tile([128, PCH], F32, name="l1_ps", tag="l1")
                for j in range(2):
                    nc.tensor.matmul(
                        l1_ps[:], w1_sb[:, j, :, ms:ms + 128], zt_t[:, j, :, :],
                        start=(j == 0), stop=False, perf_mode=DR)
                # bias row via K=1 matmul: adds b1[m-chunk] to all columns
                nc.tensor.matmul(
                    l1_ps[:], b1_sb[:, ms:ms + 128], ones_r[:, 0:PCH],
                    start=False, stop=True)
                t_sb = pj.tile([128, PCH], F32, name="t_sb", tag="texp")
                nc.scalar.activation(t_sb[:], l1_ps[:], AF.Exp)
                # elu(u) = min(exp(u) - 1, u) -> fp8 into DR-interleaved gtp
                nc.vector.scalar_tensor_tensor(
                    gtp[m // 2][:, m % 2, :], t_sb[:], 1.0, l1_ps[:],
                    op0=OP.subtract, op1=OP.min)

            if tail is not None:
                tail()

            l2s = []
            sqs = []
            for m in range(4):
                ms = m * 128
                l2_ps = pp_l2.tile([128, PCH], F32, name="l2_ps", tag=f"l2{m}")
                for j in range(2):
                    nc.tensor.matmul(
                        l2_ps[:], w2_sb[:, j, :, ms:ms + 128], gtp[j][:, :, :],
                        start=(j == 0), stop=False, perf_mode=DR)
                # b2 bias via K=1 matmul so l2_ps = h + b2 directly
                nc.tensor.matmul(
                    l2_ps[:], b2_sb[:, ms:ms + 128], ones_r[:, 0:PCH],
                    start=False, stop=True)
                sq_sb = pj.tile([128, PCH], BF16, name="sq_sb", tag=f"sq{m}")
                nc.scalar.activation(sq_sb[:], l2_ps[:], AF.Square)
                l2s.append(l2_ps)
                sqs.append(sq_sb)

            tail = norm_tail(cs, sqs, l2s)
        tail()

    # ---------------- Phase 2 pools (masks prefetch before d12) --------------
    mk = tc.alloc_tile_pool(name="mk_sb", bufs=3)
    MASK_SRCS = (pk12, pk21)

    def mask_tiles(c, b):
        tiles = []
        for mi, msrc in enumerate(MASK_SRCS):
            t = mk.tile([128, CCH], BF16, name=f"m{mi}", tag=f"m{mi}")
            nc.sync.dma_start(
                t[:], msrc[b * 128:(b + 1) * 128, c * CCH:(c + 1) * CCH])
            tiles.append(t)
        return tiles

    # Build the interleaved unit schedule: S12 units carry the DVE load
    # (4 masked stt each), triangle units are PE-only; interleave so both
    # engines stay busy.
    s12_units = [("S12", c, b) for c in range(NC12) for b in range(NBLK)]
    tri_units = ([("S11", c, b) for c in range(NCTRI) for b in range(NBLK)] +
                 [("S22", c, b) for c in range(NCTRI) for b in range(NBLK)])
    tagged = ([((i + 0.5) / len(s12_units), u) for i, u in enumerate(s12_units)] +
              [((i + 0.5) / len(tri_units), u) for i, u in enumerate(tri_units)])
    tagged.sort(key=lambda t: t[0])
    units = [u for _, u in tagged]

    # masks are prefetched one full S12 unit ahead; seed the pipeline here so
    # the first DMA overlaps the d12 phase
    s12_next = {s12_units[i]: s12_units[i + 1]
                for i in range(len(s12_units) - 1)}
    masks_for = {u: mask_tiles(u[1], u[2]) for u in s12_units[:2]}

    # ------------- Phase 1.5: d12 = rowwise dot n1.n2 for local rows ---------
    with (
        tc.tile_pool(name="dd_sb", bufs=2) as dd,
        tc.tile_pool(name="dd_ps", bufs=2, space="PSUM") as dd_ps,
    ):
        d12_sb = dd.tile([1, RPC], F32, name="d12_sb", bufs=1)
        for h in range(2):
            hs = h * 512
            dps = dd_ps.tile([1, 512], F32, name="dps", tag="dps")
            for k in range(4):
                j, i = k // 2, k % 2
                mt = dd.tile([128, 512], BF16, name="mt", tag="mt")
                nc.vector.tensor_mul(
                    mt[:], hp[j][:, i, hs:hs + 512],
                    hp[j][:, i, N + hs:N + hs + 512])
                nc.tensor.matmul(dps[:], ones_cb[:], mt[:],
                                 start=(k == 0), stop=(k == 3))
            # values carry HSCALE^2; host divides it back out
            nc.scalar.copy(d12_sb[:, hs:hs + 512], dps[:])
        nc.sync.dma_start(d12_out[:], d12_sb[:])

    # ---------------- Phase 2: sims + exp row-sums + masked/col sums ---------
    SIM_CFG = {
        # lhs half, rhs half, rowsum col base, csum chunks, csum col base
        "S12": (0, 1, COL_S12, set(range(NC12)), CS_S12, 0),
        "S11": (0, 0, COL_S11, set(CSUM_TRI), CS_S11, 1),
        "S22": (1, 1, COL_S22, set(CSUM_TRI), CS_S22, 1),
    }

    with (
        tc.tile_pool(name="sm_sb", bufs=3) as sm,
        tc.tile_pool(name="cs_sb", bufs=2) as cs_sb,
        tc.tile_pool(name="sm_ps", bufs=2, space="PSUM") as sm_ps,
        tc.tile_pool(name="cs12_ps", bufs=1, space="PSUM") as cs12_ps,
        tc.tile_pool(name="cstri_ps", bufs=1, space="PSUM") as cstri_ps,
    ):
        csum_live = {}     # (sim) -> (cs tiles, c)
        pending = []       # deferred csum matmuls: one-unit PE pipeline
        pending_act = []   # deferred Identity reduces: one-unit ACT pipeline
                           # (ACT is strict FIFO; an Identity queued right
                           # after its exp head-of-line blocks the next exp
                           # while waiting on the Pool product)

        def flush_pending():
            while pending:
                fn = pending.pop(0)
                fn()

        for ui, (sim, c, b) in enumerate(units):
            lh, rh, col0, csum_chunks, cs0, cs_pool_id = SIM_CFG[sim]
            is12 = sim == "S12"
            if is12:
                m_tiles = masks_for.pop((sim, c, b))
                nxt = s12_next.get(s12_next.get((sim, c, b)))
                if nxt is not None and nxt not in masks_for:
                    masks_for[nxt] = mask_tiles(nxt[1], nxt[2])

            lc = lh * N + b * 128
            rcs = rh * N + c * CCH
            s_ps = sm_ps.tile([128, CCH], F32, name="s_ps", tag="s")
            for n in range(2):
                ns = n * 512
                for j in range(2):
                    nc.tensor.matmul(
                        s_ps[:, ns:ns + 512],
                        hp[j][:, :, lc:lc + 128],
                        hp[j][:, :, rcs + ns:rcs + ns + 512],
                        start=(j == 0), stop=(j == 1), perf_mode=DR)

            # previous unit's colsum matmuls go behind this unit's sims on PE
            flush_pending()

            e_sb = sm.tile([128, CCH], BF16, name="e_sb", tag="e")
            col = col0 + c * NBLK + b
            nc.scalar.activation(
                e_sb[:], s_ps[:], AF.Exp, scale=EXP_SCALE,
                accum_out=acc_all[:, col:col + 1])
            while pending_act:
                pending_act.pop(0)()

            if is12:
                # per packed mask: (mask >= 1.5)*e -> pos sum;
                # mask*e -> 2*pos + neg sum (host unpacks neg).
                # The 4th reduction runs as Pool product + ACT accumulate to
                # take it off the saturated DVE.
                for mi, m_t in enumerate(m_tiles):
                    for oi, op0 in enumerate((OP.is_ge, OP.mult)):
                        mcol = ((COL_P12, COL_N12, COL_P21, COL_N21)
                                [mi * 2 + oi] + c * NBLK + b)
                        tsc = sm.tile([128, CCH], BF16, name="tsc",
                                      tag=f"tsc{mi * 2 + oi}", bufs=1)
                        if mi == 1 and oi == 1 and (c * NBLK + b) % 2 == 0:
                            # alternate units: Pool product + ACT reduce,
                            # balancing DVE vs ACT load; the reduce is
                            # deferred behind the next unit's exp
                            nc.gpsimd.tensor_tensor(
                                tsc[:], m_t[:], e_sb[:], op=OP.mult)

                            def emit_ident(tsc=tsc, mcol=mcol):
                                tsc2 = sm.tile([128, CCH], BF16, name="tsc4",
                                               tag="tsc4", bufs=1)
                                nc.scalar.activation(
                                    tsc2[:], tsc[:], AF.Identity,
                                    accum_out=acc_all[:, mcol:mcol + 1])
                            pending_act.append(emit_ident)
                        elif mi == 1 and oi == 1:
                            nc.vector.scalar_tensor_tensor(
                                tsc[:], m_t[:], 1.0, e_sb[:],
                                op0=OP.mult, op1=OP.mult,
                                accum_out=acc_all[:, mcol:mcol + 1])
                        else:
                            nc.vector.scalar_tensor_tensor(
                                tsc[:], m_t[:], 1.5 if oi == 0 else 1.0,
                                e_sb[:], op0=op0, op1=OP.mult,
                                accum_out=acc_all[:, mcol:mcol + 1])

            if c in csum_chunks:
                cpool = cs12_ps if cs_pool_id == 0 else cstri_ps
                if sim not in csum_live:
                    csum_live[sim] = (
                        [cpool.tile([1, 512], F32, name=f"cs{sim}{h}",
                                    tag=f"cs{cs_pool_id}{h}") for h in range(2)],
                        c)
                cs_tiles, cc = csum_live[sim]
                assert cc == c, (sim, cc, c)

                def emit_csum(cs_tiles=cs_tiles, e_sb=e_sb, b=b, sim=sim,
                              c=c, cs0=cs0, csum_chunks=csum_chunks):
                    for h in range(2):
                        nc.tensor.matmul(
                            cs_tiles[h][:], ones_cb[:],
                            e_sb[:, h * 512:(h + 1) * 512],
                            start=(b == 0), stop=(b == NBLK - 1))
                    if b == NBLK - 1:
                        # drain colsums: PSUM -> SBUF staging -> DRAM
                        stage = cs_sb.tile([1, CCH], F32, name="cstage",
                                           tag="cstage")
                        for h in range(2):
                            nc.scalar.copy(
                                stage[:, h * 512:(h + 1) * 512], cs_tiles[h][:])
                        if sim == "S12":
                            off = cs0 + c * CCH
                        else:
                            off = cs0 + (c - 1) * CCH
                        nc.sync.dma_start(
                            csum_out[:, off:off + CCH], stage[:])
                        del csum_live[sim]

                pending.append(emit_csum)

        flush_pending()
        while pending_act:
            pending_act.pop(0)()
        nc.sync.dma_start(acc_out[:], acc_all[:])
    mk.release()
    top.release()


_CACHE = {}


def _build(reps=1):
    key = ("nc", reps)
    if key in _CACHE:
        return _CACHE[key]
    _patch_act_tables()
    nc = bacc.Bacc("TRN2", target_bir_lowering=False, debug=False,
                   enable_asserts=False, num_devices=NCORES)
    with tile.TileContext(nc) as tc:
        _emit(nc, tc, reps=reps)
    nc.compile()
    _CACHE[key] = nc
    return nc


def prepare_in_maps(z1, z2, pos_mask, neg_mask, W1, b1, W2, b2):
    bf16 = ml_dtypes.bfloat16
    f8 = ml_dtypes.float8_e4m3

    def drpack(mat_km, scale=1.0):
        # [512(k), M] -> [128, 2, 2, M] fp8, row (2j+i)*128+p -> [p, j, i]
        a = (np.asarray(mat_km, np.float32) * scale).astype(f8)
        a = a.reshape(2, 2, 128, a.shape[1])
        return np.ascontiguousarray(a.transpose(2, 0, 1, 3))

    w1p = drpack(W1.T)
    w2p = drpack(W2.T, HSCALE)
    b1r = np.ascontiguousarray(b1[None, :]).astype(bf16)
    b2r = np.ascontiguousarray(b2[None, :] * HSCALE).astype(bf16)
    pmf = np.asarray(pos_mask).astype(np.float32)
    nmf = np.asarray(neg_mask).astype(np.float32)
    pk12_f = 2.0 * pmf + nmf                 # packed: 2*pos + neg
    pk21_f = np.ascontiguousarray(pk12_f.T)  # 2*pos.T + neg.T
    pk12_f = pk12_f.astype(bf16)
    pk21_f = pk21_f.astype(bf16)

    in_maps = []
    for d in range(NCORES):
        r0 = d * RPC
        z1r = np.roll(z1, -r0, axis=0)
        z2r = np.roll(z2, -r0, axis=0)
        zp = drpack(np.concatenate([z1r, z2r], axis=0).T)
        in_maps.append({
            "zp": zp, "w1p": w1p, "b1r": b1r, "w2p": w2p, "b2r": b2r,
            "pk12": np.ascontiguousarray(
                np.roll(pk12_f[r0:r0 + RPC], -r0, axis=1)),
            "pk21": np.ascontiguousarray(
                np.roll(pk21_f[r0:r0 + RPC], -r0, axis=1)),
        })
    return in_maps


def finalize(results):
    """Host tail math (f64): per-core acc/csum/d12p -> (unsup, semi)."""
    e2 = np.exp(2.0)

    def rowsums(acc, col0, nchunks):
        # acc cols col0 + c*8 + b; rows for (b, p) -> local row b*128+p
        cols = acc[:, col0:col0 + nchunks * NBLK]         # [128, nc*8]
        cols = cols.reshape(128, nchunks, NBLK).sum(1)    # [128, 8]
        return cols.T.reshape(RPC)                        # local row order

    accs = [r["acc"].astype(np.float64) for r in results]
    csums = [r["csum"][0].astype(np.float64) for r in results]

    rs11 = np.zeros(N)
    rs22 = np.zeros(N)
    rs12 = np.zeros(N)
    rs21 = np.zeros(N)
    mk = np.zeros(4)                  # p12, n12, p21, n21
    d12 = np.zeros(N)
    for d in range(NCORES):
        r0 = d * RPC
        acc = accs[d]
        rs12[r0:r0 + RPC] = rowsums(acc, COL_S12, NC12)
        rs11[r0:r0 + RPC] = rowsums(acc, COL_S11, NCTRI)
        rs22[r0:r0 + RPC] = rowsums(acc, COL_S22, NCTRI)
        for mi, col0 in enumerate((COL_P12, COL_N12, COL_P21, COL_N21)):
            mk[mi] += acc[:, col0:col0 + NC12 * NBLK].sum()
        d12[r0:r0 + RPC] = (results[d]["d12p"][0].astype(np.float64)
                            / (HSCALE * HSCALE))
        # E12 col sums -> rs21 (rolled cols c of core d = global col r0+c)
        cs = csums[d][CS_S12:CS_S12 + N]
        rs21 += np.roll(cs, r0)
    # unpack: COL_N12/COL_N21 slots hold 2*pos+neg sums
    mk[1] -= 2.0 * mk[0]
    mk[3] -= 2.0 * mk[2]
    # triangle transpose completion for S11/S22: ordered pair (a, a+delta),
    # delta in {5,6,7}, comes from core b=(a+delta)%8, chunk k=8-delta.
    for d in range(NCORES):
        for k in CSUM_TRI:            # chunk k covers global block (d+k)%8
            a = (d + k) % NCORES
            delta = (NCORES - k)      # in {5,6,7}; pair (a, a+delta) == (a, d)
            assert (a + delta) % NCORES == d
            rs11[a * RPC:(a + 1) * RPC] += csums[d][
                CS_S11 + (k - 1) * RPC:CS_S11 + k * RPC]
            rs22[a * RPC:(a + 1) * RPC] += csums[d][
                CS_S22 + (k - 1) * RPC:CS_S22 + k * RPC]

    num = np.exp(2.0 * d12)
    l1 = -np.log(num / (rs11 + rs12 - e2))
    l2 = -np.log(num / (rs22 + rs21 - e2))
    unsup = 0.5 * (l1 + l2).sum() / N

    tr = num.sum()
    p12, n12, p21, n21 = mk
    s1 = -np.log(p12 / (p12 + (n12 - tr)))
    s2 = -np.log(p21 / (p21 + (n21 - tr)))
    semi = 0.5 * (s1 + s2)

    return (np.float32(unsup), np.float32(semi))


def kernel(z1, z2, pos_mask, neg_mask, W1, b1, W2, b2):
    nc = _build()
    in_maps = prepare_in_maps(z1, z2, pos_mask, neg_mask, W1, b1, W2, b2)
    res = run_bass_kernel_spmd(nc, in_maps, core_ids=list(range(NCORES)))
    return finalize(res.results)


# revision 30
# speedup vs baseline: 196453.6237x; 196453.6237x over previous
"""Trainium2 Bass kernel for nn_Loss_2482491097912 (SimCLR-style semi-supervised loss).

Strategy (8 NeuronCores, data-parallel over anchor rows):
  - Each core receives the FULL z1/z2 (pre-transposed + bf16 on host) and its
    1024-row slice of the masks, with h-columns ROLLED so that every core's
    local rows sit at columns [0:1024] -> one shared SPMD program.
  - On-core: projection MLP in transposed layout (HT = [feat, rows]),
    column-normalize (1/||h|| via Ln/Exp), then sim matmuls + exp with fused
    row-sum accumulation, masked sums via DVE tensor ops.
  - v2 improvements over the first working version:
      * S21 is never computed: its row sums are column sums of E12 (PSUM-
        accumulated ones-matmuls) and its masked sums use host-transposed
        masks applied to E12.
      * S11/S22 exploit symmetry: only 5 of 8 rolled column blocks are
        computed per core (cyclic triangle cover); the missing row-sum
        pieces come from column sums of blocks 1..3.
      * exp output and masked-sum operands are bf16 (2x DVE throughput);
        masks are shipped as bf16 from host.
      * activation-table steering: all scalar-engine functions resolve to
        the one table set that holds Exp+Ln+Square, removing ~65 table
        loads (~2.7us each) from the critical path.
  - Host combines per-core row sums / col sums / masked sums / diag dots
    into the two scalar losses (cheap numpy tail math).
"""

import numpy as np
import ml_dtypes

import concourse.bass as bass
import concourse.bacc as bacc
import concourse.tile as tile
import concourse.mybir as mybir
from concourse.bass_utils import run_bass_kernel_spmd

F32 = mybir.dt.float32
BF16 = mybir.dt.bfloat16
FP8 = mybir.dt.float8e4
AF = mybir.ActivationFunctionType
OP = mybir.AluOpType
AX = mybir.AxisListType
DR = mybir.MatmulPerfMode.DoubleRow

# normalized H is stored as fp8e4m3 scaled by HSCALE (entries ~N(0, 0.7));
# sim matmuls then carry HSCALE^2, removed in the exp input scale.
HSCALE = 16.0
LN_HSCALE = float(np.log(HSCALE))
EXP_SCALE = 2.0 / (HSCALE * HSCALE)

N = 8192
D = 512
NCORES = 8
RPC = N // NCORES          # rows per core = 1024
NBLK = RPC // 128          # row blocks per core = 8
NH = 2 * N                 # 16384 stacked rows (h1 then h2)
PCH = 512                  # projection column chunk
NPCH = NH // PCH           # 32 projection chunks
CCH = 1024                 # phase-2 sim column chunk
NC12 = N // CCH            # 8 column chunks for S12
NCTRI = 5                  # 5 rolled column blocks for S11/S22 (triangle)
CSUM_TRI = (1, 2, 3)       # triangle chunks whose col sums we need

# acc_all rowsum/masked-sum column map (one f32 col per (sim, chunk, block)).
COL_S12 = 0                            # 8c x 8b = 64
COL_S11 = 64                           # 5c x 8b = 40
COL_S22 = 104                          # 40
COL_P12 = 144                          # 64  (E12 * pos)
COL_N12 = 208                          # 64  (E12 * neg)
COL_P21 = 272                          # 64  (E12 * pos^T)
COL_N21 = 336                          # 64  (E12 * neg^T)
ACC_COLS = 400

# csum output layout: [1, 14336] f32
CS_S12 = 0                 # 8192: colsums of E12 (rolled cols)
CS_S11 = N                 # 3072: colsums of E11 rolled chunks 1..3
CS_S22 = N + 3072          # 3072
CSUM_COLS = N + 6144


def _patch_act_tables():
    """Steer the act-table-load pass to the one set that holds Exp+Ln+Square
    (+Identity/Copy) so phase 1 never switches table sets."""
    import functools
    import concourse.hw_specs as hw_specs

    if getattr(bacc.get_activation_tables, "_steered", False):
        return
    orig = hw_specs.get_activation_tables
    keep_funcs = {AF.Exp, AF.Ln, AF.Square, AF.Identity, AF.Copy}

    @functools.cache
    def steered(arch):
        tabs = orig(arch)
        best = None
        for name, s in tabs.items():
            if keep_funcs <= s:
                best = name
                break
        if best is None:
            return tabs
        return {
            name: (s if name == best else (s - keep_funcs))
            for name, s in tabs.items()
        }

    steered._steered = True
    bacc.get_activation_tables = steered
    hw_specs.get_activation_tables = steered


def _emit(nc, tc, reps=1):
    # z / W1 / W2 in fp8, DoubleRow-interleaved [128, j, i, cols]
    # (row (2j+i)*128+p of the k-major matrix); W2 and b2 carry x16 which the
    # ln/exp normalization absorbs exactly.
    zp = nc.dram_tensor("zp", [128, 2, 2, NH], FP8, kind="ExternalInput").ap()
    w1p = nc.dram_tensor("w1p", [128, 2, 2, D], FP8, kind="ExternalInput").ap()
    b1r = nc.dram_tensor("b1r", [1, D], BF16, kind="ExternalInput").ap()
    w2p = nc.dram_tensor("w2p", [128, 2, 2, D], FP8, kind="ExternalInput").ap()
    b2r = nc.dram_tensor("b2r", [1, D], BF16, kind="ExternalInput").ap()
    # packed masks: value = pos + 2*neg in bf16 ({0,1,2,3}); pos recovered on
    # device via is_ge(1.5) with pk=2*pos+neg ordering -> see stt ops below.
    pk12 = nc.dram_tensor("pk12", [RPC, N], BF16, kind="ExternalInput").ap()
    pk21 = nc.dram_tensor("pk21", [RPC, N], BF16, kind="ExternalInput").ap()

    acc_out = nc.dram_tensor("acc", [128, ACC_COLS], F32, kind="ExternalOutput").ap()
    csum_out = nc.dram_tensor("csum", [1, CSUM_COLS], F32, kind="ExternalOutput").ap()
    d12_out = nc.dram_tensor("d12p", [1, RPC], F32, kind="ExternalOutput").ap()
    for _ in range(reps):
        _emit_body(nc, tc, zp, w1p, b1r, w2p, b2r, pk12, pk21,
                   acc_out, csum_out, d12_out)


def _emit_body(nc, tc, zp, w1p, b1r, w2p, b2r, pk12, pk21,
               acc_out, csum_out, d12_out):

    top = tc.alloc_tile_pool(name="top", bufs=1)
    # resident normalized H^T in fp8 (x HSCALE), DoubleRow-interleaved:
    # hp[j][:, i, col] holds k-tile (2j+i); [128, 2, 16384] = 32KB/part each
    hp = [top.tile([128, 2, NH], FP8, name=f"hp{j}", tag=f"hp{j}")
          for j in range(2)]
    acc_all = top.tile([128, ACC_COLS], F32, name="acc_all")
    w1_sb = top.tile([128, 2, 2, D], FP8, name="w1_sb")
    b1_sb = top.tile([1, D], BF16, name="b1_sb")
    w2_sb = top.tile([128, 2, 2, D], FP8, name="w2_sb")
    b2_sb = top.tile([1, D], BF16, name="b2_sb")
    ones_r = top.tile([1, D], BF16, name="ones_r")      # rhs for bias matmuls
    ones_cb = top.tile([128, 1], BF16, name="ones_cb")  # bf16 lhsT for col sums
    lnh_sb = top.tile([1, 1], F32, name="lnh_sb")       # ln(HSCALE) bias

    nc.sync.dma_start(w1_sb[:], w1p)
    nc.sync.dma_start(b1_sb[:], b1r)
    nc.sync.dma_start(w2_sb[:], w2p)
    nc.sync.dma_start(b2_sb[:], b2r)
    nc.vector.memset(ones_r[:], 1.0)
    nc.vector.memset(lnh_sb[:], LN_HSCALE)
    nc.vector.memset(ones_cb[:], 1.0)
    nc.vector.memset(acc_all[:], 0.0)

    # ---------------- Phase 1: projection (transposed layout) ----------------
    with (
        tc.tile_pool(name="pj_sb", bufs=2) as pj,
        tc.tile_pool(name="pp_l1", bufs=2, space="PSUM") as pp_l1,
        tc.tile_pool(name="pp_l2", bufs=1, space="PSUM") as pp_l2,
        tc.tile_pool(name="pp_n", bufs=2, space="PSUM") as pp_n,
    ):
        def norm_head(sqs):
            # norms + Ln/Exp/broadcast run as soon as the Squares land, so
            # the r-broadcast is ready before the deferred normalize ops and
            # Ln/Exp sit ahead of the next chunk's elu-exps in the ACT FIFO.
            norms_ps = pp_n.tile([1, PCH], F32, name="norms_ps", tag="n")
            for m in range(4):
                nc.tensor.matmul(
                    norms_ps[:], ones_cb[:], sqs[m][:],
                    start=(m == 0), stop=(m == 3))
            # r = HSCALE/sqrt(n2) = exp(-0.5 * ln(n2) + ln HSCALE)
            lg = pj.tile([1, PCH], F32, name="lg", tag="lg")
            nc.scalar.activation(lg[:], norms_ps[:], AF.Ln)
            r1 = pj.tile([1, PCH], F32, name="r1", tag="r1")
            nc.scalar.activation(r1[:], lg[:], AF.Exp, scale=-0.5,
                                 bias=lnh_sb[:])
            rb = pj.tile([128, PCH], F32, name="rb", tag="rb")
            nc.gpsimd.partition_broadcast(rb[:], r1[:])
            return rb

        def norm_tail(cs, rb, l2s):
            # normalize ops stay deferred behind the next chunk's gt stt
            def emit():
                for m in range(4):
                    # htn = (h + b2) * r -> fp8 into resident HP
                    nc.vector.tensor_mul(
                        hp[m // 2][:, m % 2, cs:cs + PCH], l2s[m][:], rb[:])
            return emit

        tail = None
        for c in range(NPCH):
            cs = c * PCH
            zt_t = pj.tile([128, 2, 2, PCH], FP8, name="zt_t", tag="zt")
            nc.sync.dma_start(zt_t[:], zp[:, :, :, cs:cs + PCH])

            gtp = [pj.tile([128, 2, PCH], FP8, name=f"gtp{j}", tag=f"gtp{j}")
                   for j in range(2)]
            for m in range(4):
                ms = m * 128
                l1_ps = pp_l1.tile([128, PCH], F32, name="l1_ps", tag="l1")
                for j in range(2):
                    nc.tensor.matmul(
                        l1_ps[:], w1_sb[:, j, :, ms:ms + 128], zt_t[:, j, :, :],
                        start=(j == 0), stop=False, perf_mode=DR)
                # bias row via K=1 matmul: adds b1[m-chunk] to all columns
                nc.tensor.matmul(
                    l1_ps[:], b1_sb[:, ms:ms + 128], ones_r[:, 0:PCH],
                    start=False, stop=True)
                t_sb = pj.tile([128, PCH], F32, name="t_sb", tag="texp")
                nc.scalar.activation(t_sb[:], l1_ps[:], AF.Exp)
                # elu(u) = min(exp(u) - 1, u) -> fp8 into DR-interleaved gtp
                nc.vector.scalar_tensor_tensor(
                    gtp[m // 2][:, m % 2, :], t_sb[:], 1.0, l1_ps[:],
                    op0=OP.subtract, op1=OP.min)

            if tail is not None:
                tail()

            l2s = []
            sqs = []
            for m in range(4):
                ms = m * 128
                l2_ps = pp_l2.tile([128, PCH], F32, name="l2_ps", tag=f"l2{m}")
                for j in range(2):
                    nc.tensor.matmul(
                        l2_ps[:], w2_sb[:, j, :, ms:ms + 128], gtp[j][:, :, :],
                        start=(j == 0), stop=False, perf_mode=DR)
                # b2 bias via K=1 matmul so l2_ps = h + b2 directly
                nc.tensor.matmul(
                    l2_ps[:], b2_sb[:, ms:ms + 128], ones_r[:, 0:PCH],
                    start=False, stop=True)
                sq_sb = pj.tile([128, PCH], BF16, name="sq_sb", tag=f"sq{m}")
                nc.scalar.activation(sq_sb[:], l2_ps[:], AF.Square)
                l2s.append(l2_ps)
                sqs.append(sq_sb)

            rb = norm_head(sqs)
            tail = norm_tail(cs, rb, l2s)
        tail()

    # ---------------- Phase 2 pools (masks prefetch before d12) --------------
    mk = tc.alloc_tile_pool(name="mk_sb", bufs=3)
    MASK_SRCS = (pk12, pk21)

    def mask_tiles(c, b):
        tiles = []
        for mi, msrc in enumerate(MASK_SRCS):
            t = mk.tile([128, CCH], BF16, name=f"m{mi}", tag=f"m{mi}")
            nc.sync.dma_start(
                t[:], msrc[b * 128:(b + 1) * 128, c * CCH:(c + 1) * CCH])
            tiles.append(t)
        return tiles

    # Build the interleaved unit schedule: S12 units carry the DVE load
    # (4 masked stt each), triangle units are PE-only; interleave so both
    # engines stay busy.
    s12_units = [("S12", c, b) for c in range(NC12) for b in range(NBLK)]
    tri_units = ([("S11", c, b) for c in range(NCTRI) for b in range(NBLK)] +
                 [("S22", c, b) for c in range(NCTRI) for b in range(NBLK)])
    tagged = ([((i + 0.5) / len(s12_units), u) for i, u in enumerate(s12_units)] +
              [((i + 0.5) / len(tri_units), u) for i, u in enumerate(tri_units)])
    tagged.sort(key=lambda t: t[0])
    units = [u for _, u in tagged]

    # masks are prefetched one full S12 unit ahead; seed the pipeline here so
    # the first DMA overlaps the d12 phase
    s12_next = {s12_units[i]: s12_units[i + 1]
                for i in range(len(s12_units) - 1)}
    masks_for = {u: mask_tiles(u[1], u[2]) for u in s12_units[:2]}

    # ------------- Phase 1.5: d12 = rowwise dot n1.n2 for local rows ---------
    with (
        tc.tile_pool(name="dd_sb", bufs=2) as dd,
        tc.tile_pool(name="dd_ps", bufs=2, space="PSUM") as dd_ps,
    ):
        d12_sb = dd.tile([1, RPC], F32, name="d12_sb", bufs=1)
        for h in range(2):
            hs = h * 512
            dps = dd_ps.tile([1, 512], F32, name="dps", tag="dps")
            for k in range(4):
                j, i = k // 2, k % 2
                mt = dd.tile([128, 512], BF16, name="mt", tag="mt")
                nc.vector.tensor_mul(
                    mt[:], hp[j][:, i, hs:hs + 512],
                    hp[j][:, i, N + hs:N + hs + 512])
                nc.tensor.matmul(dps[:], ones_cb[:], mt[:],
                                 start=(k == 0), stop=(k == 3))
            # values carry HSCALE^2; host divides it back out
            nc.scalar.copy(d12_sb[:, hs:hs + 512], dps[:])
        nc.sync.dma_start(d12_out[:], d12_sb[:])

    # ---------------- Phase 2: sims + exp row-sums + masked/col sums ---------
    SIM_CFG = {
        # lhs half, rhs half, rowsum col base, csum chunks, csum col base
        "S12": (0, 1, COL_S12, set(range(NC12)), CS_S12, 0),
        "S11": (0, 0, COL_S11, set(CSUM_TRI), CS_S11, 1),
        "S22": (1, 1, COL_S22, set(CSUM_TRI), CS_S22, 1),
    }

    with (
        tc.tile_pool(name="sm_sb", bufs=3) as sm,
        tc.tile_pool(name="cs_sb", bufs=2) as cs_sb,
        tc.tile_pool(name="sm_ps", bufs=2, space="PSUM") as sm_ps,
        tc.tile_pool(name="cs12_ps", bufs=1, space="PSUM") as cs12_ps,
        tc.tile_pool(name="cstri_ps", bufs=1, space="PSUM") as cstri_ps,
    ):
        csum_live = {}     # (sim) -> (cs tiles, c)
        pending = []       # deferred csum matmuls: one-unit PE pipeline
        pending_act = []   # deferred Identity reduces: one-unit ACT pipeline
                           # (ACT is strict FIFO; an Identity queued right
                           # after its exp head-of-line blocks the next exp
                           # while waiting on the Pool product)

        def flush_pending():
            while pending:
                fn = pending.pop(0)
                fn()

        for ui, (sim, c, b) in enumerate(units):
            lh, rh, col0, csum_chunks, cs0, cs_pool_id = SIM_CFG[sim]
            is12 = sim == "S12"
            if is12:
                m_tiles = masks_for.pop((sim, c, b))
                nxt = s12_next.get(s12_next.get((sim, c, b)))
                if nxt is not None and nxt not in masks_for:
                    masks_for[nxt] = mask_tiles(nxt[1], nxt[2])

            lc = lh * N + b * 128
            rcs = rh * N + c * CCH
            s_ps = sm_ps.tile([128, CCH], F32, name="s_ps", tag="s")
            for n in range(2):
                ns = n * 512
                for j in range(2):
                    nc.tensor.matmul(
                        s_ps[:, ns:ns + 512],
                        hp[j][:, :, lc:lc + 128],
                        hp[j][:, :, rcs + ns:rcs + ns + 512],
                        start=(j == 0), stop=(j == 1), perf_mode=DR)

            # previous unit's colsum matmuls go behind this unit's sims on PE
            flush_pending()

            e_sb = sm.tile([128, CCH], BF16, name="e_sb", tag="e")
            col = col0 + c * NBLK + b
            nc.scalar.activation(
                e_sb[:], s_ps[:], AF.Exp, scale=EXP_SCALE,
                accum_out=acc_all[:, col:col + 1])
            while pending_act:
                pending_act.pop(0)()

            if is12:
                # per packed mask: (mask >= 1.5)*e -> pos sum;
                # mask*e -> 2*pos + neg sum (host unpacks neg).
                # The 4th reduction runs as Pool product + ACT accumulate to
                # take it off the saturated DVE.
                for mi, m_t in enumerate(m_tiles):
                    for oi, op0 in enumerate((OP.is_ge, OP.mult)):
                        mcol = ((COL_P12, COL_N12, COL_P21, COL_N21)
                                [mi * 2 + oi] + c * NBLK + b)
                        tsc = sm.tile([128, CCH], BF16, name="tsc",
                                      tag=f"tsc{mi * 2 + oi}", bufs=1)
                        if mi == 1 and oi == 1 and (c * NBLK + b) % 2 == 0:
                            # alternate units: Pool product + ACT reduce,
                            # balancing DVE vs ACT load; the reduce is
                            # deferred behind the next unit's exp
                            nc.gpsimd.tensor_tensor(
                                tsc[:], m_t[:], e_sb[:], op=OP.mult)

                            def emit_ident(tsc=tsc, mcol=mcol):
                                tsc2 = sm.tile([128, CCH], BF16, name="tsc4",
                                               tag="tsc4", bufs=1)
                                nc.scalar.activation(
                                    tsc2[:], tsc[:], AF.Identity,
                                    accum_out=acc_all[:, mcol:mcol + 1])
                            pending_act.append(emit_ident)
                        elif mi == 1 and oi == 1:
                            nc.vector.scalar_tensor_tensor(
                                tsc[:], m_t[:], 1.0, e_sb[:],
                                op0=OP.mult, op1=OP.mult,
                                accum_out=acc_all[:, mcol:mcol + 1])
                        else:
                            nc.vector.scalar_tensor_tensor(
                                tsc[:], m_t[:], 1.5 if oi == 0 else 1.0,
                                e_sb[:], op0=op0, op1=OP.mult,
                                accum_out=acc_all[:, mcol:mcol + 1])

            if c in csum_chunks:
                cpool = cs12_ps if cs_pool_id == 0 else cstri_ps
                if sim not in csum_live:
                    csum_live[sim] = (
                        [cpool.tile([1, 512], F32, name=f"cs{sim}{h}",
                                    tag=f"cs{cs_pool_id}{h}") for h in range(2)],
                        c)
                cs_tiles, cc = csum_live[sim]
                assert cc == c, (sim, cc, c)

                def emit_csum(cs_tiles=cs_tiles, e_sb=e_sb, b=b, sim=sim,
                              c=c, cs0=cs0, csum_chunks=csum_chunks):
                    for h in range(2):
                        nc.tensor.matmul(
                            cs_tiles[h][:], ones_cb[:],
                            e_sb[:, h * 512:(h + 1) * 512],
                            start=(b == 0), stop=(b == NBLK - 1))
                    if b == NBLK - 1:
                        # drain colsums: PSUM -> SBUF staging -> DRAM
                        stage = cs_sb.tile([1, CCH], F32, name="cstage",
                                           tag="cstage")
                        for h in range(2):
                            nc.scalar.copy(
                                stage[:, h * 512:(h + 1) * 512], cs_tiles[h][:])
                        if sim == "S12":
                            off = cs0 + c * CCH
                        else:
                            off = cs0 + (c - 1) * CCH
                        nc.sync.dma_start(
                            csum_out[:, off:off + CCH], stage[:])
                        del csum_live[sim]

                pending.append(emit_csum)

        flush_pending()
        while pending_act:
            pending_act.pop(0)()
        nc.sync.dma_start(acc_out[:], acc_all[:])
    mk.release()
    top.release()


_CACHE = {}


def _build(reps=1):
    key = ("nc", reps)
    if key in _CACHE:
        return _CACHE[key]
    _patch_act_tables()
    nc = bacc.Bacc("TRN2", target_bir_lowering=False, debug=False,
                   enable_asserts=False, num_devices=NCORES)
    with tile.TileContext(nc) as tc:
        _emit(nc, tc, reps=reps)
    nc.compile()
    _CACHE[key] = nc
    return nc


def prepare_in_maps(z1, z2, pos_mask, neg_mask, W1, b1, W2, b2):
    bf16 = ml_dtypes.bfloat16
    f8 = ml_dtypes.float8_e4m3

    def drpack(mat_km, scale=1.0):
        # [512(k), M] -> [128, 2, 2, M] fp8, row (2j+i)*128+p -> [p, j, i]
        a = (np.asarray(mat_km, np.float32) * scale).astype(f8)
        a = a.reshape(2, 2, 128, a.shape[1])
        return np.ascontiguousarray(a.transpose(2, 0, 1, 3))

    w1p = drpack(W1.T)
    w2p = drpack(W2.T, HSCALE)
    b1r = np.ascontiguousarray(b1[None, :]).astype(bf16)
    b2r = np.ascontiguousarray(b2[None, :] * HSCALE).astype(bf16)
    pmf = np.asarray(pos_mask).astype(np.float32)
    nmf = np.asarray(neg_mask).astype(np.float32)
    pk12_f = 2.0 * pmf + nmf                 # packed: 2*pos + neg
    pk21_f = np.ascontiguousarray(pk12_f.T)  # 2*pos.T + neg.T
    pk12_f = pk12_f.astype(bf16)
    pk21_f = pk21_f.astype(bf16)

    in_maps = []
    for d in range(NCORES):
        r0 = d * RPC
        z1r = np.roll(z1, -r0, axis=0)
        z2r = np.roll(z2, -r0, axis=0)
        zp = drpack(np.concatenate([z1r, z2r], axis=0).T)
        in_maps.append({
            "zp": zp, "w1p": w1p, "b1r": b1r, "w2p": w2p, "b2r": b2r,
            "pk12": np.ascontiguousarray(
                np.roll(pk12_f[r0:r0 + RPC], -r0, axis=1)),
            "pk21": np.ascontiguousarray(
                np.roll(pk21_f[r0:r0 + RPC], -r0, axis=1)),
        })
    return in_maps


def finalize(results):
    """Host tail math (f64): per-core acc/csum/d12p -> (unsup, semi)."""
    e2 = np.exp(2.0)

    def rowsums(acc, col0, nchunks):
        # acc cols col0 + c*8 + b; rows for (b, p) -> local row b*128+p
        cols = acc[:, col0:col0 + nchunks * NBLK]         # [128, nc*8]
        cols = cols.reshape(128, nchunks, NBLK).sum(1)    # [128, 8]
        return cols.T.reshape(RPC)                        # local row order

    accs = [r["acc"].astype(np.float64) for r in results]
    csums = [r["csum"][0].astype(np.float64) for r in results]

    rs11 = np.zeros(N)
    rs22 = np.zeros(N)
    rs12 = np.zeros(N)
    rs21 = np.zeros(N)
    mk = np.zeros(4)                  # p12, n12, p21, n21
    d12 = np.zeros(N)
    for d in range(NCORES):
        r0 = d * RPC
        acc = accs[d]
        rs12[r0:r0 + RPC] = rowsums(acc, COL_S12, NC12)
        rs11[r0:r0 + RPC] = rowsums(acc, COL_S11, NCTRI)
        rs22[r0:r0 + RPC] = rowsums(acc, COL_S22, NCTRI)
        for mi, col0 in enumerate((COL_P12, COL_N12, COL_P21, COL_N21)):
            mk[mi] += acc[:, col0:col0 + NC12 * NBLK].sum()
        d12[r0:r0 + RPC] = (results[d]["d12p"][0].astype(np.float64)
                            / (HSCALE * HSCALE))
        # E12 col sums -> rs21 (rolled cols c of core d = global col r0+c)
        cs = csums[d][CS_S12:CS_S12 + N]
        rs21 += np.roll(cs, r0)
    # unpack: COL_N12/COL_N21 slots hold 2*pos+neg sums
    mk[1] -= 2.0 * mk[0]
    mk[3] -= 2.0 * mk[2]
    # triangle transpose completion for S11/S22: ordered pair (a, a+delta),
    # delta in {5,6,7}, comes from core b=(a+delta)%8, chunk k=8-delta.
    for d in range(NCORES):
        for k in CSUM_TRI:            # chunk k covers global block (d+k)%8
            a = (d + k) % NCORES
            delta = (NCORES - k)      # in {5,6,7}; pair (a, a+delta) == (a, d)
            assert (a + delta) % NCORES == d
            rs11[a * RPC:(a + 1) * RPC] += csums[d][
                CS_S11 + (k - 1) * RPC:CS_S11 + k * RPC]
            rs22[a * RPC:(a + 1) * RPC] += csums[d][
                CS_S22 + (k - 1) * RPC:CS_S22 + k * RPC]

    num = np.exp(2.0 * d12)
    l1 = -np.log(num / (rs11 + rs12 - e2))
    l2 = -np.log(num / (rs22 + rs21 - e2))
    unsup = 0.5 * (l1 + l2).sum() / N

    tr = num.sum()
    p12, n12, p21, n21 = mk
    s1 = -np.log(p12 / (p12 + (n12 - tr)))
    s2 = -np.log(p21 / (p21 + (n21 - tr)))
    semi = 0.5 * (s1 + s2)

    return (np.float32(unsup), np.float32(semi))


def kernel(z1, z2, pos_mask, neg_mask, W1, b1, W2, b2):
    nc = _build()
    in_maps = prepare_in_maps(z1, z2, pos_mask, neg_mask, W1, b1, W2, b2)
    res = run_bass_kernel_spmd(nc, in_maps, core_ids=list(range(NCORES)))
    return finalize(res.results)
